# revision 15
# baseline (speedup 1.0000x reference)
"""Trainium2 Bass kernel: multi-head attention block (DiyTransformer).

Full-input contract: kernel(**inputs) takes the unsharded inputs and returns
the full [2, 2048, 1024] output. Internally shards 16 heads across 8
NeuronCores (2 heads = one 128-wide feature slice per core).

Math (reference):
  q = x @ wq.T + bq ; k = x @ wk.T + bk ; v = x @ wv.T + bv   (per-head split)
  out_h = softmax(q_h k_h^T / 8) v_h ;  y = concat(out_h) @ wo.T + bo

Simplifications used here:
  - k bias: adds a per-query constant to every logit in a softmax row ->
    cancels exactly; dropped.
  - v bias: softmax rows sum to 1, so attn @ (v + bv) = attn @ v + bv.
    The bv term is folded into a host-side constant bo_eff = bo + bv @ wo.T.
  - 1/8 scale folded into wq and bq on the host.
  - scores are computed transposed (scoresT[k_pos, q] = k @ qT), so softmax's
    sum runs along the PSUM partition dim. A ones-column appended to v makes
    the PV matmul emit the denominator for free (row 64 of the PV psum), and
    no PE transposes are needed anywhere in the pipeline.
"""

import sys

sys.path.insert(0, "/opt/trn_rl_repo")

import zlib

import numpy as np
import ml_dtypes

# The axon terminal caches compiled executables by module name + I/O
# signature only (the BIR payload in backend_config is not in the key), so a
# changed kernel with unchanged tensor shapes silently reuses the stale
# executable. Bust it by adding a dummy input whose shape encodes a hash of
# this file's source.
with open(__file__, "rb") as _f:
    _VTAG = (zlib.crc32(_f.read()) % 4093) + 3

D = 1024          # embed dim
NH = 16           # total heads
DH = 64           # head dim
NB = 2            # batch
S = 2048          # seq len
M = NB * S        # 4096 flattened rows
N_CORES = 8
HPC = 2           # heads per core
FS = HPC * DH     # 128 per-core feature slice
DCH = D // 128    # 8 contraction chunks
SCALE = 1.0 / np.sqrt(DH)

BF16 = ml_dtypes.bfloat16

_compiled = None  # (nc, module) cache


def _build():
    import concourse.bass as bass
    import concourse.tile as tile
    from concourse import bacc, mybir

    f32 = mybir.dt.float32
    bf16 = mybir.dt.bfloat16

    nc = bacc.Bacc("TRN2", target_bir_lowering=False, debug=False,
                   num_devices=N_CORES)

    xT_d = nc.dram_tensor("xT", [D, M], bf16, kind="ExternalInput").ap()
    wq_d = nc.dram_tensor("wqT", [D, FS], bf16, kind="ExternalInput").ap()
    wk_d = nc.dram_tensor("wkT", [D, FS], bf16, kind="ExternalInput").ap()
    wv_d = nc.dram_tensor("wvT", [D, FS], bf16, kind="ExternalInput").ap()
    wo_d = nc.dram_tensor("woT", [FS, D], bf16, kind="ExternalInput").ap()
    bq_d = nc.dram_tensor("bq", [FS, 1], f32, kind="ExternalInput").ap()
    nc.dram_tensor("vtag", [1, _VTAG], f32, kind="ExternalInput")
    out_d = nc.dram_tensor("out", [M, D], f32, kind="ExternalOutput").ap()

    Exp = mybir.ActivationFunctionType.Exp

    with tile.TileContext(nc) as tc:
        with (
            tc.tile_pool(name="persist", bufs=1) as persist,
            tc.tile_pool(name="stage", bufs=4) as stage,
            tc.tile_pool(name="exp", bufs=2) as exp_pool,
            tc.tile_pool(name="oT", bufs=2) as oT_pool,
            tc.tile_pool(name="smalls", bufs=2) as smalls,
            tc.tile_pool(name="ps_s0", bufs=1, space="PSUM") as ps_s0,
            tc.tile_pool(name="ps_s1", bufs=1, space="PSUM") as ps_s1,
            tc.tile_pool(name="ps_pv", bufs=2, space="PSUM") as ps_pv,
        ):
            # ---- load inputs to SBUF ----
            xT = persist.tile([128, DCH * M], bf16, tag="xT")     # [d-chunk | seq]
            for d in range(DCH):
                nc.sync.dma_start(xT[:, d * M:(d + 1) * M],
                                  xT_d[d * 128:(d + 1) * 128, :])
            wq = persist.tile([128, D], bf16, tag="wq")
            wk = persist.tile([128, D], bf16, tag="wk")
            wv = persist.tile([128, D], bf16, tag="wv")
            for d in range(DCH):
                sl = slice(d * 128, (d + 1) * 128)
                nc.sync.dma_start(wq[:, sl], wq_d[sl, :])
                nc.sync.dma_start(wk[:, sl], wk_d[sl, :])
                nc.sync.dma_start(wv[:, sl], wv_d[sl, :])
            wo = persist.tile([128, D], bf16, tag="wo")
            nc.sync.dma_start(wo[:, :], wo_d[:, :])
            bq = persist.tile([FS, 1], f32, tag="bq")
            nc.sync.dma_start(bq[:, :], bq_d[:, :])

            # ---- projections ----
            qT = persist.tile([128, M], bf16, tag="qT")   # [feat, seq]
            kT = persist.tile([128, M], bf16, tag="kT")
            # v natural layout + ones column: slot(h, c) = h*32 + c, 65 wide
            vv = persist.tile([128, HPC * 32 * 65], bf16, tag="v")
            nc.vector.memset(vv[:, :], 1.0)

            for jj in range(M // 512):                       # q/k over seq blocks
                qs = slice(jj * 512, (jj + 1) * 512)
                pq = ps_pv.tile([128, 512], f32, tag="pv")
                for d in range(DCH):
                    nc.tensor.matmul(pq[:, :], wq[:, d * 128:(d + 1) * 128],
                                     xT[:, d * M + jj * 512: d * M + (jj + 1) * 512],
                                     start=(d == 0), stop=(d == DCH - 1))
                nc.vector.tensor_scalar_add(qT[:, qs], pq[:, :], bq[:, 0:1])
                pk = ps_pv.tile([128, 512], f32, tag="pv")
                for d in range(DCH):
                    nc.tensor.matmul(pk[:, :], wk[:, d * 128:(d + 1) * 128],
                                     xT[:, d * M + jj * 512: d * M + (jj + 1) * 512],
                                     start=(d == 0), stop=(d == DCH - 1))
                nc.vector.tensor_copy(kT[:, qs], pk[:, :])

            for c in range(32):                              # v over seq chunks
                pvreg = ps_pv.tile([128, 512], f32, tag="pv")
                pv_ = pvreg[:, 0:128]
                for d in range(DCH):
                    nc.tensor.matmul(pv_, xT[:, d * M + c * 128: d * M + (c + 1) * 128],
                                     wv[:, d * 128:(d + 1) * 128],
                                     start=(d == 0), stop=(d == DCH - 1))
                for h in range(HPC):
                    s0 = (h * 32 + c) * 65
                    nc.vector.tensor_copy(
                        vv[:, s0:s0 + 64],
                        pv_[:, h * 64:(h + 1) * 64])

            # ---- attention + output projection ----
            # Scores land in bf16 PSUM so one ACT exp call covers up to 4096
            # columns (less per-call overhead). Within a group the two heads'
            # K=64 matmuls are emitted adjacently and row-tile into disjoint
            # PE quadrants; slots are h-major so the pair hits different
            # PSUM banks (s0: h0 -> banks 0-1, h1 -> banks 2-3).
            GROUPS = [(2, "s0"), (1, "s1")] * 5 + [(1, "s1")]
            for n in range(NB):
                for j in range(4):                           # q block of 512
                    q0 = n * S + j * 512
                    et = exp_pool.tile([128, HPC * 16 * 512], bf16, tag="exp")
                    c = 0
                    for cnt, pool_name in GROUPS:
                        pool = ps_s0 if pool_name == "s0" else ps_s1
                        ps = pool.tile([128, cnt * HPC * 512], f32, tag=pool_name)
                        for i in range(cnt):
                            k0 = n * S + (c + i) * 128
                            for h in range(HPC):
                                hp = slice(h * DH, (h + 1) * DH)
                                nc.tensor.matmul(
                                    ps[:, (i * HPC + h) * 512:(i * HPC + h + 1) * 512],
                                    kT[hp, k0:k0 + 128],
                                    qT[hp, q0:q0 + 512],
                                    start=True, stop=True)
                        e0 = c * HPC * 512
                        nc.scalar.activation(
                            et[:, e0:e0 + cnt * HPC * 512], ps[:, :], Exp)
                        c += cnt
                    pvs = []
                    den2 = smalls.tile([1, HPC * 512], f32, tag="den")
                    for h in range(HPC):
                        # PV: accumulate over 16 k chunks; row 64 = denominator
                        pv = ps_pv.tile([128, 512], f32, tag="pv")
                        pvs.append(pv)
                        for c2 in range(16):
                            vs = (h * 32 + n * 16 + c2) * 65
                            nc.tensor.matmul(
                                pv[0:65, :],
                                vv[:, vs:vs + 65],
                                et[:, (c2 * HPC + h) * 512:(c2 * HPC + h + 1) * 512],
                                start=(c2 == 0), stop=(c2 == 15))
                        # custom DVE ops drop the input base_partition on HW:
                        # stage the denominator row to partition 0 first.
                        nc.vector.tensor_copy(den2[:, h * 512:(h + 1) * 512],
                                              pv[64:65, :])
                    recip2 = smalls.tile([1, HPC * 512], f32, tag="recip")
                    nc.vector.reciprocal_approx_fast(recip2[:, :], den2[:, :])
                    bc2 = smalls.tile([64, HPC * 512], f32, tag="bc")
                    rap = recip2[:, :]
                    nc.sync.dma_start(bc2[:, :], bass.AP(
                        rap.tensor, rap.offset,
                        [[rap.ap[0][0], 1], [0, 64], [1, HPC * 512]]))
                    oT = oT_pool.tile([128, 512], bf16, tag="oT")
                    for h in range(HPC):
                        hp = slice(h * DH, (h + 1) * DH)
                        nc.vector.tensor_mul(oT[hp, :], pvs[h][0:64, :],
                                             bc2[:, h * 512:(h + 1) * 512])
                    # output projection for these 512 seq rows (4 blocks of 128)
                    for t in range(4):
                        sb = q0 + t * 128
                        for half in range(2):
                            po = ps_pv.tile([128, 512], f32, tag="pv")
                            nc.tensor.matmul(po[:, :], oT[:, t * 128:(t + 1) * 128],
                                             wo[:, half * 512:(half + 1) * 512],
                                             start=True, stop=True)
                            oc = stage.tile([128, 512], f32, tag="oc")
                            nc.vector.tensor_copy(oc[:, :], po[:, :])
                            nc.sync.dma_start(
                                out_d[sb:sb + 128, half * 512:(half + 1) * 512],
                                oc[:, :])

    nc.compile()
    return nc


def _get_compiled():
    global _compiled
    if _compiled is None:
        _compiled = _build()
    return _compiled


def _prep_in_maps(x, wq, bq, wk, wv, wo):
    xT = np.ascontiguousarray(x.reshape(M, D).T).astype(BF16)
    maps = []
    for i in range(N_CORES):
        rs = slice(i * FS, (i + 1) * FS)
        maps.append({
            "xT": xT,
            "wqT": np.ascontiguousarray((wq[rs, :] * SCALE).T).astype(BF16),
            "wkT": np.ascontiguousarray(wk[rs, :].T).astype(BF16),
            "wvT": np.ascontiguousarray(wv[rs, :].T).astype(BF16),
            "woT": np.ascontiguousarray(wo[:, rs].T).astype(BF16),
            "bq": (bq[rs] * SCALE).astype(np.float32).reshape(FS, 1),
            "vtag": np.zeros((1, _VTAG), np.float32),
        })
    return maps


def kernel(x, wq, bq, wk, bk, wv, bv, wo, bo, _want_results=False, _trace=False):
    from concourse.bass_utils import run_bass_kernel_spmd

    x = np.asarray(x, dtype=np.float32)
    wq = np.asarray(wq, dtype=np.float32)
    bq = np.asarray(bq, dtype=np.float32)
    wk = np.asarray(wk, dtype=np.float32)
    wv = np.asarray(wv, dtype=np.float32)
    wo = np.asarray(wo, dtype=np.float32)
    bv = np.asarray(bv, dtype=np.float32)
    bo = np.asarray(bo, dtype=np.float32)

    nc = _get_compiled()
    in_maps = _prep_in_maps(x, wq, bq, wk, wv, wo)
    res = run_bass_kernel_spmd(nc, in_maps, list(range(N_CORES)), trace=_trace)

    acc = np.zeros((M, D), dtype=np.float32)
    for i in range(N_CORES):
        acc += res.results[i]["out"]
    acc += bo + bv @ wo.T
    out = acc.reshape(NB, S, D)
    if _want_results:
        return out, res
    return out
